# revision 14
# baseline (speedup 1.0000x reference)
"""Trainium2 Bass kernel for the per-gene sparse-decoder MLP.

Math (per gene g): h = selu(features[:, tf_idx[g]] @ W1[g].T); h = selu(h @ Wm[i,g].T) x2;
out[:, g] = h @ Wf[g].  Genes are independent -> shard G=20000 over 8 cores (2500 each).

Layout per core (gene dim padded 2500 -> 2560, 10 "supers" of 256 genes):
  Activations live as [node-rows on partitions, batch on free].  SELU is computed
  as two streams accumulated by the next layer's matmul pair in PSUM:
      selu(z) = A + B,  A = lam*relu(z)                      (ACT Relu)
                        B = min(lam*alp*e^z, lam*alp) - lam*alp
                          = (E min lam*alp) + (-lam*alp)     (DVE tensor_scalar)
      with E = lam*alp*e^z = Exp(z + ln(lam*alp))            (ACT Exp)
  GPSIMD is deliberately unused: HW-measured tensor_scalar there is ~7.5us/op
  (16x the cost model), and it shares an SBUF port with DVE.

Schedule: LAYER-MAJOR within each super -- PE does all 8 L1 banks, then all 8
L2 banks, all 8 L3 banks, then 2 Lf banks, rotating the 8 PSUM banks as one
pool.  Every cross-engine semaphore wait is satisfied ~a full phase in
advance, so engines stream without round-trip stalls (the previous
unit-interleaved schedule serialized on cross-engine wake-ups).

The first-layer gather features[:, tf_idx] is compile-time data movement; it
is laid out on the host into the exact [pack, row, batch] bf16 tiles the
TensorEngine streams.
"""

import sys
import numpy as np

if "/opt/trn_rl_repo" not in sys.path:
    sys.path.insert(0, "/opt/trn_rl_repo")

import ml_dtypes

BF16 = ml_dtypes.bfloat16

G, W, K, T, B, D = 20000, 8, 16, 1500, 256, 2
NCORES = 8
GC = G // NCORES            # 2500 genes per core
GP = 2560                   # padded genes per core
NP = GP // 8                # 320 L1 packs
NG = GP // 16               # 160 16-gene groups
NU = GP // 32               # 80 units (32 genes)
NW = NU // 2                # 40 output windows (64 genes each)
NS = NU // 8                # 10 supers (8 units each)
NC = 2 * NS                 # 20 output chunks ([128, 512] = 2 windows)

LAM = 1.0507009873554805
ALPHA = 1.6732632423543772
LA = LAM * ALPHA
C0 = float(np.log(LA))      # Exp bias: exp(z + C0) = LA * e^z

XB = 3                      # xg stream buffer depth (supers)
ED = 8                      # e-tile buffer depth (banks)

_CACHE = {}


def _build(reps=1):
    import concourse.bass as bass
    import concourse.mybir as mybir

    f32 = mybir.dt.float32
    bf16 = mybir.dt.bfloat16
    Alu = mybir.AluOpType
    Act = mybir.ActivationFunctionType

    nc = bass.Bass()

    def reg_const(value, dtype=f32):
        t = nc.alloc_sbuf_tensor(f"const-{dtype.name}-{value}", [128, 1], dtype)
        nc.gpsimd.memset(t.ap(), value)
        nc.const_aps.aps[(dtype, value)] = t.ap()

    reg_const(C0)
    nc.all_engine_barrier()

    xg_d = nc.declare_dram_parameter("xg", [NS, 128, 32, 256], bf16, isOutput=False)
    w1_d = nc.declare_dram_parameter("w1", [NS, 128, 32, 64], bf16, isOutput=False)
    wm2_d = nc.declare_dram_parameter("wm2", [NS, 128, 16, 128], bf16, isOutput=False)
    wm3_d = nc.declare_dram_parameter("wm3", [NS, 128, 16, 128], bf16, isOutput=False)
    wf_d = nc.declare_dram_parameter("wf", [NS, 128, 16, 16], bf16, isOutput=False)
    out_d = nc.declare_dram_parameter("out", [NC, 128, 512], f32, isOutput=True)

    TS = NS * reps           # total supers

    # ---------------- global schedule plan ----------------
    # PE fill sequence: per super: 8xL1, 8xL2, 8xL3, 2xLf  (26 fills/super)
    # Each fill targets PSUM bank (fill_idx % 8).
    fills = []               # (kind, s, u_or_chunk)
    for ss in range(TS):
        for u in range(8):
            fills.append(("L1", ss, u))
        for u in range(8):
            fills.append(("L2", ss, u))
        for u in range(8):
            fills.append(("L3", ss, u))
        for c in range(2):
            fills.append(("Lf", ss, c))
    ipe = {f: n + 1 for n, f in enumerate(fills)}      # pe_sem after fill f

    # ACT op order: for each z-fill (L1/L2/L3) in fill order: E then A.
    act_ops = []
    for f in fills:
        if f[0] != "Lf":
            act_ops.append(("E", f))
            act_ops.append(("A", f))
    iact = {op: n + 1 for n, op in enumerate(act_ops)}

    # DVE op order: B per z-fill, evac per Lf fill, in fill order.
    dve_ops = []
    for f in fills:
        if f[0] == "Lf":
            dve_ops.append(("V", f))
        else:
            dve_ops.append(("B", f))
    idve = {op: n + 1 for n, op in enumerate(dve_ops)}

    # z-fill index (for e-slot rotation) and per-fill bank
    zfills = [f for f in fills if f[0] != "Lf"]
    izf = {f: n for n, f in enumerate(zfills)}
    bank_of = {f: n % 8 for n, f in enumerate(fills)}
    # chunk index for Lf fills / output
    lffills = [f for f in fills if f[0] == "Lf"]
    ichunk = {f: n for n, f in enumerate(lffills)}

    # DMA plan: per super 5 input DMAs (xg,w1,wm2,wm3,wf) -> w_sem += 80
    # output: per super 2 chunks -> o_sem += 16 each
    from contextlib import ExitStack
    with ExitStack() as ctx:
        block = ctx.enter_context(nc.Block())

        def sb(name, shape, dt=bf16):
            return ctx.enter_context(nc.sbuf_tensor(name, shape, dt))

        xg_sb = sb("xg_sb", [128, XB, 32, 256])
        w1_sb = sb("w1_sb", [128, 2, 32, 64])
        wm2_sb = sb("wm2_sb", [128, 2, 16, 128])
        wm3_sb = sb("wm3_sb", [128, 2, 16, 128])
        wf_sb = sb("wf_sb", [128, 2, 16, 16])
        e_sb = sb("e_sb", [128, ED, 512])
        a_sb = [sb(f"a{l}_sb", [128, 8, 512]) for l in (1, 2, 3)]
        b_sb = [sb(f"b{l}_sb", [128, 8, 512]) for l in (1, 2, 3)]
        o_sb = sb("o_sb", [128, 6, 512], f32)
        banks = [ctx.enter_context(nc.psum_tensor(f"zb{i}", [128, 512], f32))
                 for i in range(8)]

        pe_sem = ctx.enter_context(nc.semaphore("pe_sem"))
        act_sem = ctx.enter_context(nc.semaphore("act_sem"))
        dve_sem = ctx.enter_context(nc.semaphore("dve_sem"))
        # parity-alternating DMA sems: a sem may only take increments from one
        # DMA batch at a time across a waited threshold (sim race detector)
        w_sems = (ctx.enter_context(nc.semaphore("w_sem0")),
                  ctx.enter_context(nc.semaphore("w_sem1")))
        o_sems = (ctx.enter_context(nc.semaphore("o_sem0")),
                  ctx.enter_context(nc.semaphore("o_sem1")))

        def ab(layer):
            return a_sb[layer - 1], b_sb[layer - 1]

        @block.sync
        def _(sync):
            for ss in range(TS + 2):
                if ss < TS:
                    s = ss % NS
                    ws = w_sems[ss % 2]
                    # previous same-parity super's inputs fully landed
                    if ss >= 2:
                        sync.wait_ge(ws, 80 * (ss // 2))
                    # xg slot ss%XB free once PE finished L1 of super ss-XB
                    if ss >= XB:
                        sync.wait_ge(pe_sem, ipe[("L1", ss - XB, 7)])
                    sync.dma_start(out=xg_sb[:, ss % XB], in_=xg_d[s]).then_inc(ws, 16)
                    # weight slots ss%2 free once super ss-2 consumed them
                    if ss >= 2:
                        sync.wait_ge(pe_sem, ipe[("Lf", ss - 2, 1)])
                    sync.dma_start(out=w1_sb[:, ss % 2], in_=w1_d[s]).then_inc(ws, 16)
                    sync.dma_start(out=wm2_sb[:, ss % 2], in_=wm2_d[s]).then_inc(ws, 16)
                    sync.dma_start(out=wm3_sb[:, ss % 2], in_=wm3_d[s]).then_inc(ws, 16)
                    sync.dma_start(out=wf_sb[:, ss % 2], in_=wf_d[s]).then_inc(ws, 16)
                if ss >= 2:
                    # outputs of super ss-2
                    for c in range(2):
                        q = 2 * (ss - 2) + c
                        os_ = o_sems[q % 2]
                        if q >= 2:
                            sync.wait_ge(os_, 16 * (q // 2))
                        sync.wait_ge(dve_sem, idve[("V", ("Lf", ss - 2, c))])
                        sync.dma_start(out=out_d[q % NC], in_=o_sb[:, q % 6]).then_inc(os_, 16)
            sync.wait_ge(o_sems[0], 16 * TS)
            sync.wait_ge(o_sems[1], 16 * TS)

        @block.tensor
        def _(tensor):
            for fi, f in enumerate(fills):
                kind, ss, x = f
                s = ss % NS
                j = ss % 2
                bank = banks[bank_of[f]]
                # bank free: drain of fill fi-8 complete
                if fi >= 8:
                    prev = fills[fi - 8]
                    if prev[0] == "Lf":
                        tensor.wait_ge(dve_sem, idve[("V", prev)])
                    else:
                        tensor.wait_ge(act_sem, iact[("A", prev)])
                if kind == "L1":
                    if x == 0:
                        # all 5 input DMAs of super ss landed (per-DMA count
                        # thresholds are racy: SDMA engines interleave incs)
                        tensor.wait_ge(w_sems[ss % 2], 80 * (ss // 2 + 1))
                    for m in range(4):
                        lp = 4 * x + m
                        mm = tensor.matmul(
                            bank[(m % 2) * 64:(m % 2) * 64 + 64,
                                 (m // 2) * 256:(m // 2) * 256 + 256],
                            w1_sb[:, j, lp, :],
                            xg_sb[:, ss % XB, lp, :],
                            start=True, stop=True,
                            tile_position=(0, (m % 2) * 64),
                        )
                    mm.then_inc(pe_sem, 1)
                elif kind in ("L2", "L3"):
                    layer = 2 if kind == "L2" else 3
                    wsb = wm2_sb if kind == "L2" else wm3_sb
                    asrc, bsrc = ab(layer - 1)
                    # h tile x of the previous layer ready (per-bank, so the
                    # next phase starts as soon as its first input is drained)
                    feed = (("L1" if kind == "L2" else "L2"), ss, x)
                    tensor.wait_ge(act_sem, iact[("A", feed)])
                    tensor.wait_ge(dve_sem, idve[("B", feed)])
                    for gg in range(2):
                        ln = 2 * x + gg
                        sl = slice(gg * 256, gg * 256 + 256)
                        tensor.matmul(bank[:, sl], wsb[:, j, ln, :],
                                      bsrc[:, x, sl], start=True, stop=False)
                        mm = tensor.matmul(bank[:, sl], wsb[:, j, ln, :],
                                           asrc[:, x, sl], start=False, stop=True)
                    mm.then_inc(pe_sem, 1)
                else:  # Lf chunk x (2 windows, units 4x..4x+3)
                    asrc, bsrc = ab(3)
                    feed = ("L3", ss, 4 * x + 3)
                    tensor.wait_ge(act_sem, iact[("A", feed)])
                    tensor.wait_ge(dve_sem, idve[("B", feed)])
                    for h in range(2):          # window within chunk
                        v = 2 * x + h           # window within super
                        for n in range(2):      # unit within window
                            u = 2 * v + n
                            for gg in range(2):
                                strip = 2 * n + gg
                                ln = 2 * u + gg
                                sl = slice(gg * 256, gg * 256 + 256)
                                zout = bank[strip * 32:strip * 32 + 16,
                                            h * 256:h * 256 + 256]
                                tensor.matmul(zout, wf_sb[:, j, ln, :],
                                              bsrc[:, u, sl], start=True, stop=False,
                                              tile_position=(0, strip * 32))
                                mm = tensor.matmul(zout, wf_sb[:, j, ln, :],
                                                   asrc[:, u, sl], start=False, stop=True,
                                                   tile_position=(0, strip * 32))
                    mm.then_inc(pe_sem, 1)

        @block.scalar
        def _(scalar):
            for op, f in act_ops:
                kind, ss, u = f
                layer = {"L1": 1, "L2": 2, "L3": 3}[kind]
                asrc, bsrc = ab(layer)
                bank = banks[bank_of[f]]
                k = izf[f]
                if op == "E":
                    scalar.wait_ge(pe_sem, ipe[f])
                    if k >= ED:
                        # e-slot free once B of fill k-ED consumed it
                        scalar.wait_ge(dve_sem, idve[("B", zfills[k - ED])])
                    scalar.activation(e_sb[:, k % ED, :], bank[:], Act.Exp,
                                      bias=C0, scale=1.0).then_inc(act_sem, 1)
                else:
                    scalar.activation(asrc[:, u, :], bank[:], Act.Relu,
                                      bias=0.0, scale=LAM).then_inc(act_sem, 1)

        @block.vector
        def _(vector):
            for op, f in dve_ops:
                kind, ss, x = f
                if op == "B":
                    layer = {"L1": 1, "L2": 2, "L3": 3}[kind]
                    asrc, bsrc = ab(layer)
                    k = izf[f]
                    vector.wait_ge(act_sem, iact[("E", f)])
                    vector.tensor_scalar(bsrc[:, x, :], e_sb[:, k % ED, :], LA, -LA,
                                         Alu.min, Alu.add).then_inc(dve_sem, 1)
                else:  # evac
                    q = ichunk[f]
                    vector.wait_ge(pe_sem, ipe[f])
                    if q >= 6:
                        # o_sb slot free once out-DMA q-6 (same parity) done
                        vector.wait_ge(o_sems[q % 2], 16 * ((q - 6) // 2 + 1))
                    vector.tensor_copy(o_sb[:, q % 6], banks[bank_of[f]][:],
                                       ).then_inc(dve_sem, 1)

    return nc


def _prepare_core_inputs(features, tf_idx, W1, Wm, Wf):
    """Host-side layout: gather + block-diagonal packing, all bf16."""
    fbf = features.astype(BF16)
    maps = []
    for c in range(NCORES):
        g0 = c * GC
        tf_l = np.zeros((GP, K), np.int64)
        tf_l[:GC] = tf_idx[g0:g0 + GC]
        W1_l = np.zeros((GP, W, K), np.float32)
        W1_l[:GC] = W1[g0:g0 + GC]
        Wm_l = np.zeros((D, GP, W, W), np.float32)
        Wm_l[:, :GC] = Wm[:, g0:g0 + GC]
        Wf_l = np.zeros((GP, W), np.float32)
        Wf_l[:GC] = Wf[g0:g0 + GC]

        # xg: [NS, 128, 32, 256]  row q=16j+k of pack p = features[:, tf[8p+j, k]]
        gath = fbf[:, tf_l.reshape(-1)]                     # [B, GP*K] bf16
        xg = np.ascontiguousarray(gath.T).reshape(NP, 128, 256)
        xg = np.ascontiguousarray(
            xg.reshape(NS, 32, 128, 256).transpose(0, 2, 1, 3))

        jj = np.arange(8)
        w1b = np.zeros((NP, 8, K, 8, W), np.float32)
        w1b[:, jj, :, jj, :] = W1_l.reshape(NP, 8, W, K).transpose(
            0, 1, 3, 2).transpose(1, 0, 2, 3)
        w1b = w1b.reshape(NP, 128, 64).astype(BF16)
        w1b = np.ascontiguousarray(
            w1b.reshape(NS, 32, 128, 64).transpose(0, 2, 1, 3))

        j16 = np.arange(16)
        t4 = np.arange(4)
        wmb = []
        for l in range(D):
            t = np.zeros((NG, 16, W, 16, W), np.float32)
            t[:, j16, :, j16, :] = Wm_l[l].reshape(NG, 16, W, W).transpose(
                0, 1, 3, 2).transpose(1, 0, 2, 3)
            t = t.reshape(NG, 4, 32, 4, 32)
            # keep only the diagonal [32,32] tiles (4 genes each): 4x smaller
            d = t[:, t4, :, t4, :].transpose(1, 0, 2, 3)     # [NG, 4, 32, 32]
            d = d.reshape(NG, 128, 32).astype(BF16)
            wmb.append(np.ascontiguousarray(
                d.reshape(NS, 16, 128, 32).transpose(0, 2, 1, 3)))

        wfb = np.zeros((NG, 16, W, 16), np.float32)
        wfb[:, j16, :, j16] = Wf_l.reshape(NG, 16, W).transpose(1, 0, 2)
        wfb = wfb.reshape(NG, 128, 16).astype(BF16)
        wfb = np.ascontiguousarray(
            wfb.reshape(NS, 16, 128, 16).transpose(0, 2, 1, 3))

        maps.append({"xg": xg, "w1": w1b, "wm2": wmb[0], "wm3": wmb[1], "wf": wfb})
    return maps


def _assemble(results):
    """Per-core out [NC, 128, 512] -> full [B, G] f32.

    Chunk q holds windows v=2q (cols 0:256) and v=2q+1 (cols 256:512);
    window strip layout: partitions 32*strip..+16 = genes 64v+16*strip..+16.
    """
    out = np.empty((B, G), np.float32)
    for c, r in enumerate(results):
        oc = np.asarray(r["out"])                          # [NC, 128, 512]
        oc = oc.reshape(NC, 4, 32, 2, 256)[:, :, :16, :, :]  # [q, strip, j, h, b]
        genes = oc.transpose(0, 3, 1, 2, 4).reshape(GP, 256)[:GC]
        out[:, c * GC:(c + 1) * GC] = genes.T
    return out


def kernel(features, tf_idx, W1, b1, Wm, bm, Wf, bf):
    from concourse.bass_utils import run_bass_kernel_spmd

    features = np.asarray(features, np.float32)
    tf_idx = np.asarray(tf_idx)
    assert not np.any(np.asarray(b1)) and not np.any(np.asarray(bm)) \
        and not np.any(np.asarray(bf)), "nonzero biases not supported"

    if "nc" not in _CACHE:
        _CACHE["nc"] = _build()
    nc = _CACHE["nc"]

    in_maps = _prepare_core_inputs(
        features, tf_idx, np.asarray(W1, np.float32),
        np.asarray(Wm, np.float32), np.asarray(Wf, np.float32))

    res = run_bass_kernel_spmd(nc, in_maps, list(range(NCORES)))
    return _assemble(res.results)


# revision 22
# speedup vs baseline: 1.1671x; 1.1671x over previous
"""Trainium2 Bass kernel for the per-gene sparse-decoder MLP.

Math (per gene g): h = selu(features[:, tf_idx[g]] @ W1[g].T); h = selu(h @ Wm[i,g].T) x2;
out[:, g] = h @ Wf[g].  Genes are independent -> shard G=20000 over 8 cores (2500 each).

Layout per core (gene dim padded 2500 -> 2560, 10 "supers" of 256 genes):
  Activations live as [node-rows on partitions, batch on free].  SELU is computed
  as two streams accumulated by the next layer's matmul pair in PSUM:
      selu(z) = A + B,  A = lam*relu(z)                      (ACT Relu)
                        B = min(lam*alp*e^z, lam*alp) - lam*alp
                          = (E min lam*alp) + (-lam*alp)     (DVE tensor_scalar)
      with E = lam*alp*e^z = Exp(z + ln(lam*alp))            (ACT Exp)
  GPSIMD is deliberately unused: HW-measured tensor_scalar there is ~7.5us/op
  (16x the cost model), and it shares an SBUF port with DVE.

Schedule: LAYER-MAJOR within each super -- PE does all 8 L1 banks, then all 8
L2 banks, all 8 L3 banks, then 2 Lf banks, rotating the 8 PSUM banks as one
pool.  Every cross-engine semaphore wait is satisfied ~a full phase in
advance, so engines stream without round-trip stalls (the previous
unit-interleaved schedule serialized on cross-engine wake-ups).

The first-layer gather features[:, tf_idx] is compile-time data movement; it
is laid out on the host into the exact [pack, row, batch] bf16 tiles the
TensorEngine streams.
"""

import sys
import numpy as np

if "/opt/trn_rl_repo" not in sys.path:
    sys.path.insert(0, "/opt/trn_rl_repo")

import ml_dtypes

BF16 = ml_dtypes.bfloat16

G, W, K, T, B, D = 20000, 8, 16, 1500, 256, 2
NCORES = 8
GC = G // NCORES            # 2500 genes per core
GP = 2560                   # padded genes per core
NP = GP // 8                # 320 L1 packs
NG = GP // 16               # 160 16-gene groups
NU = GP // 32               # 80 units (32 genes)
NW = NU // 2                # 40 output windows (64 genes each)
NS = NU // 8                # 10 supers (8 units each)
NC = 2 * NS                 # 20 output chunks ([128, 512] = 2 windows)

LAM = 1.0507009873554805
ALPHA = 1.6732632423543772
LA = LAM * ALPHA
# selu(z) = LAM * s'(z),  s'(z) = relu(z) + min(ALPHA*e^z, ALPHA) - ALPHA.
# The LAM factor is folded into the next layer's weights (wm/wf pre-scaled on
# host), so every ACT op runs with scale=1.0 (scale != 1 is a measured ~2-10x
# slower ACT path on HW).
CA = float(np.log(ALPHA))   # Exp bias: exp(z + CA) = ALPHA * e^z

XB = 3                      # xg stream buffer depth (supers)
ED = 8                      # e-tile buffer depth (banks)

_CACHE = {}
_DISABLE = set()   # timing-only ablations: subsets of {'pe','act','dve'}


def _build(reps=1):
    import concourse.bass as bass
    import concourse.mybir as mybir

    f32 = mybir.dt.float32
    bf16 = mybir.dt.bfloat16
    Alu = mybir.AluOpType
    Act = mybir.ActivationFunctionType

    nc = bass.Bass()

    def reg_const(value, dtype=f32):
        t = nc.alloc_sbuf_tensor(f"const-{dtype.name}-{value}", [128, 1], dtype)
        nc.gpsimd.memset(t.ap(), value)
        nc.const_aps.aps[(dtype, value)] = t.ap()

    reg_const(CA)
    nc.all_engine_barrier()

    xg_d = nc.declare_dram_parameter("xg", [NS, 128, 32, 256], bf16, isOutput=False)
    w1_d = nc.declare_dram_parameter("w1", [NS, 128, 32, 64], bf16, isOutput=False)
    wm2_d = nc.declare_dram_parameter("wm2", [NS, 128, 16, 128], bf16, isOutput=False)
    wm3_d = nc.declare_dram_parameter("wm3", [NS, 128, 16, 128], bf16, isOutput=False)
    wf_d = nc.declare_dram_parameter("wf", [NS, 128, 16, 16], bf16, isOutput=False)
    out_d = nc.declare_dram_parameter("out", [NC, 128, 512], f32, isOutput=True)

    TS = NS * reps           # total supers

    # ---------------- global schedule plan ----------------
    # PE fill sequence: per super: 8xL1, 8xL2, 8xL3, 2xLf  (26 fills/super)
    # Each fill targets PSUM bank (fill_idx % 8).
    fills = []               # (kind, s, u_or_chunk)
    for ss in range(TS):
        for u in range(8):
            fills.append(("L1", ss, u))
        for u in range(8):
            fills.append(("L2", ss, u))
        for u in range(8):
            fills.append(("L3", ss, u))
        for c in range(2):
            fills.append(("Lf", ss, c))
    ipe = {f: n + 1 for n, f in enumerate(fills)}      # pe_sem after fill f

    # ACT op order: for each z-fill (L1/L2/L3) in fill order: E then A.
    act_ops = []
    for f in fills:
        if f[0] != "Lf":
            act_ops.append(("E", f))
            act_ops.append(("A", f))
    iact = {op: n + 1 for n, op in enumerate(act_ops)}

    # DVE op order: B per z-fill, evac per Lf fill, in fill order.
    dve_ops = []
    for f in fills:
        if f[0] == "Lf":
            dve_ops.append(("V", f))
        else:
            dve_ops.append(("B", f))
    idve = {op: n + 1 for n, op in enumerate(dve_ops)}

    # z-fill index (for e-slot rotation) and per-fill bank
    zfills = [f for f in fills if f[0] != "Lf"]
    izf = {f: n for n, f in enumerate(zfills)}
    bank_of = {f: n % 8 for n, f in enumerate(fills)}
    # chunk index for Lf fills / output
    lffills = [f for f in fills if f[0] == "Lf"]
    ichunk = {f: n for n, f in enumerate(lffills)}

    # DMA plan: per super 5 input DMAs (xg,w1,wm2,wm3,wf) -> w_sem += 80
    # output: per super 2 chunks -> o_sem += 16 each
    from contextlib import ExitStack
    with ExitStack() as ctx:
        block = ctx.enter_context(nc.Block())

        def sb(name, shape, dt=bf16):
            return ctx.enter_context(nc.sbuf_tensor(name, shape, dt))

        xg_sb = sb("xg_sb", [128, XB, 32, 256])
        w1_sb = sb("w1_sb", [128, 2, 32, 64])
        wm2_sb = sb("wm2_sb", [128, 2, 16, 128])
        wm3_sb = sb("wm3_sb", [128, 2, 16, 128])
        wf_sb = sb("wf_sb", [128, 2, 16, 16])
        e_sb = sb("e_sb", [128, ED, 512])
        a_sb = [sb(f"a{l}_sb", [128, 8, 512]) for l in (1, 2, 3)]
        b_sb = [sb(f"b{l}_sb", [128, 8, 512]) for l in (1, 2, 3)]
        o_sb = sb("o_sb", [128, 6, 512], f32)
        banks = [ctx.enter_context(nc.psum_tensor(f"zb{i}", [128, 512], f32))
                 for i in range(8)]

        pe_sem = ctx.enter_context(nc.semaphore("pe_sem"))
        act_sem = ctx.enter_context(nc.semaphore("act_sem"))
        dve_sem = ctx.enter_context(nc.semaphore("dve_sem"))
        # parity-alternating DMA sems: a sem may only take increments from one
        # DMA batch at a time across a waited threshold (sim race detector)
        w_sems = (ctx.enter_context(nc.semaphore("w_sem0")),
                  ctx.enter_context(nc.semaphore("w_sem1")))
        o_sems = (ctx.enter_context(nc.semaphore("o_sem0")),
                  ctx.enter_context(nc.semaphore("o_sem1")))

        def ab(layer):
            return a_sb[layer - 1], b_sb[layer - 1]

        dis = frozenset(_DISABLE)

        def wait_pe(eng, thr):
            if "pe" not in dis:
                eng.wait_ge(pe_sem, thr)

        def wait_act(eng, thr):
            if "act" not in dis:
                eng.wait_ge(act_sem, thr)

        def wait_dve(eng, thr):
            if "dve" not in dis:
                eng.wait_ge(dve_sem, thr)

        @block.sync
        def _(sync):
            for ss in range(TS + 2):
                if ss < TS:
                    s = ss % NS
                    ws = w_sems[ss % 2]
                    # previous same-parity super's inputs fully landed
                    if ss >= 2:
                        sync.wait_ge(ws, 80 * (ss // 2))
                    # xg slot ss%XB free once PE finished L1 of super ss-XB
                    if ss >= XB:
                        wait_pe(sync, ipe[("L1", ss - XB, 7)])
                    sync.dma_start(out=xg_sb[:, ss % XB], in_=xg_d[s]).then_inc(ws, 16)
                    # weight slots ss%2 free once super ss-2 consumed them
                    if ss >= 2:
                        wait_pe(sync, ipe[("Lf", ss - 2, 1)])
                    sync.dma_start(out=w1_sb[:, ss % 2], in_=w1_d[s]).then_inc(ws, 16)
                    sync.dma_start(out=wm2_sb[:, ss % 2], in_=wm2_d[s]).then_inc(ws, 16)
                    sync.dma_start(out=wm3_sb[:, ss % 2], in_=wm3_d[s]).then_inc(ws, 16)
                    sync.dma_start(out=wf_sb[:, ss % 2], in_=wf_d[s]).then_inc(ws, 16)
                if ss >= 2:
                    # outputs of super ss-2
                    for c in range(2):
                        q = 2 * (ss - 2) + c
                        os_ = o_sems[q % 2]
                        if q >= 2:
                            sync.wait_ge(os_, 16 * (q // 2))
                        wait_dve(sync, idve[("V", ("Lf", ss - 2, c))])
                        sync.dma_start(out=out_d[q % NC], in_=o_sb[:, q % 6]).then_inc(os_, 16)
            sync.wait_ge(o_sems[0], 16 * TS)
            sync.wait_ge(o_sems[1], 16 * TS)

        @block.tensor
        def _(tensor):
            if "pe" in dis:
                return
            for fi, f in enumerate(fills):
                kind, ss, x = f
                s = ss % NS
                j = ss % 2
                bank = banks[bank_of[f]]
                # bank free: drain of fill fi-8 complete
                if fi >= 8:
                    prev = fills[fi - 8]
                    if prev[0] == "Lf":
                        wait_dve(tensor, idve[("V", prev)])
                    else:
                        wait_act(tensor, iact[("A", prev)])
                if kind == "L1":
                    if x == 0:
                        # all 5 input DMAs of super ss landed (per-DMA count
                        # thresholds are racy: SDMA engines interleave incs)
                        tensor.wait_ge(w_sems[ss % 2], 80 * (ss // 2 + 1))
                    for m in range(4):
                        lp = 4 * x + m
                        mm = tensor.matmul(
                            bank[(m % 2) * 64:(m % 2) * 64 + 64,
                                 (m // 2) * 256:(m // 2) * 256 + 256],
                            w1_sb[:, j, lp, :],
                            xg_sb[:, ss % XB, lp, :],
                            start=True, stop=True,
                            tile_position=(0, (m % 2) * 64),
                        )
                    mm.then_inc(pe_sem, 1)
                elif kind in ("L2", "L3"):
                    layer = 2 if kind == "L2" else 3
                    wsb = wm2_sb if kind == "L2" else wm3_sb
                    asrc, bsrc = ab(layer - 1)
                    # h tile x of the previous layer ready (per-bank, so the
                    # next phase starts as soon as its first input is drained)
                    feed = (("L1" if kind == "L2" else "L2"), ss, x)
                    wait_act(tensor, iact[("A", feed)])
                    wait_dve(tensor, idve[("B", feed)])
                    for gg in range(2):
                        ln = 2 * x + gg
                        sl = slice(gg * 256, gg * 256 + 256)
                        tensor.matmul(bank[:, sl], wsb[:, j, ln, :],
                                      bsrc[:, x, sl], start=True, stop=False)
                        mm = tensor.matmul(bank[:, sl], wsb[:, j, ln, :],
                                           asrc[:, x, sl], start=False, stop=True)
                    mm.then_inc(pe_sem, 1)
                else:  # Lf chunk x (2 windows, units 4x..4x+3)
                    asrc, bsrc = ab(3)
                    feed = ("L3", ss, 4 * x + 3)
                    wait_act(tensor, iact[("A", feed)])
                    wait_dve(tensor, idve[("B", feed)])
                    for h in range(2):          # window within chunk
                        v = 2 * x + h           # window within super
                        for n in range(2):      # unit within window
                            u = 2 * v + n
                            for gg in range(2):
                                strip = 2 * n + gg
                                ln = 2 * u + gg
                                sl = slice(gg * 256, gg * 256 + 256)
                                zout = bank[strip * 32:strip * 32 + 16,
                                            h * 256:h * 256 + 256]
                                tensor.matmul(zout, wf_sb[:, j, ln, :],
                                              bsrc[:, u, sl], start=True, stop=False,
                                              tile_position=(0, strip * 32))
                                mm = tensor.matmul(zout, wf_sb[:, j, ln, :],
                                                   asrc[:, u, sl], start=False, stop=True,
                                                   tile_position=(0, strip * 32))
                    mm.then_inc(pe_sem, 1)

        @block.scalar
        def _(scalar):
            if "act" in dis:
                return
            for op, f in act_ops:
                kind, ss, u = f
                layer = {"L1": 1, "L2": 2, "L3": 3}[kind]
                asrc, bsrc = ab(layer)
                bank = banks[bank_of[f]]
                k = izf[f]
                if op == "E":
                    wait_pe(scalar, ipe[f])
                    if k >= ED:
                        # e-slot free once B of fill k-ED consumed it
                        wait_dve(scalar, idve[("B", zfills[k - ED])])
                    scalar.activation(e_sb[:, k % ED, :], bank[:], Act.Exp,
                                      bias=CA, scale=1.0).then_inc(act_sem, 1)
                else:
                    scalar.activation(asrc[:, u, :], bank[:], Act.Relu,
                                      bias=0.0, scale=1.0).then_inc(act_sem, 1)

        @block.vector
        def _(vector):
            if "dve" in dis:
                return
            for op, f in dve_ops:
                kind, ss, x = f
                if op == "B":
                    layer = {"L1": 1, "L2": 2, "L3": 3}[kind]
                    asrc, bsrc = ab(layer)
                    k = izf[f]
                    wait_act(vector, iact[("E", f)])
                    vector.tensor_scalar(bsrc[:, x, :], e_sb[:, k % ED, :], ALPHA, -ALPHA,
                                         Alu.min, Alu.add).then_inc(dve_sem, 1)
                else:  # evac
                    q = ichunk[f]
                    wait_pe(vector, ipe[f])
                    if q >= 6:
                        # o_sb slot free once out-DMA q-6 (same parity) done
                        vector.wait_ge(o_sems[q % 2], 16 * ((q - 6) // 2 + 1))
                    vector.tensor_copy(o_sb[:, q % 6], banks[bank_of[f]][:],
                                       ).then_inc(dve_sem, 1)

    return nc


def _prepare_core_inputs(features, tf_idx, W1, Wm, Wf):
    """Host-side layout: gather + block-diagonal packing, all bf16."""
    fbf = features.astype(BF16)
    maps = []
    for c in range(NCORES):
        g0 = c * GC
        tf_l = np.zeros((GP, K), np.int64)
        tf_l[:GC] = tf_idx[g0:g0 + GC]
        W1_l = np.zeros((GP, W, K), np.float32)
        W1_l[:GC] = W1[g0:g0 + GC]
        Wm_l = np.zeros((D, GP, W, W), np.float32)
        Wm_l[:, :GC] = Wm[:, g0:g0 + GC] * np.float32(LAM)
        Wf_l = np.zeros((GP, W), np.float32)
        Wf_l[:GC] = Wf[g0:g0 + GC] * np.float32(LAM)

        # xg: [NS, 128, 32, 256]  row q=16j+k of pack p = features[:, tf[8p+j, k]]
        gath = fbf[:, tf_l.reshape(-1)]                     # [B, GP*K] bf16
        xg = np.ascontiguousarray(gath.T).reshape(NP, 128, 256)
        xg = np.ascontiguousarray(
            xg.reshape(NS, 32, 128, 256).transpose(0, 2, 1, 3))

        jj = np.arange(8)
        w1b = np.zeros((NP, 8, K, 8, W), np.float32)
        w1b[:, jj, :, jj, :] = W1_l.reshape(NP, 8, W, K).transpose(
            0, 1, 3, 2).transpose(1, 0, 2, 3)
        w1b = w1b.reshape(NP, 128, 64).astype(BF16)
        w1b = np.ascontiguousarray(
            w1b.reshape(NS, 32, 128, 64).transpose(0, 2, 1, 3))

        j16 = np.arange(16)
        t4 = np.arange(4)
        wmb = []
        for l in range(D):
            t = np.zeros((NG, 16, W, 16, W), np.float32)
            t[:, j16, :, j16, :] = Wm_l[l].reshape(NG, 16, W, W).transpose(
                0, 1, 3, 2).transpose(1, 0, 2, 3)
            t = t.reshape(NG, 128, 128).astype(BF16)
            wmb.append(np.ascontiguousarray(
                t.reshape(NS, 16, 128, 128).transpose(0, 2, 1, 3)))

        wfb = np.zeros((NG, 16, W, 16), np.float32)
        wfb[:, j16, :, j16] = Wf_l.reshape(NG, 16, W).transpose(1, 0, 2)
        wfb = wfb.reshape(NG, 128, 16).astype(BF16)
        wfb = np.ascontiguousarray(
            wfb.reshape(NS, 16, 128, 16).transpose(0, 2, 1, 3))

        maps.append({"xg": xg, "w1": w1b, "wm2": wmb[0], "wm3": wmb[1], "wf": wfb})
    return maps


def _assemble(results):
    """Per-core out [NC, 128, 512] -> full [B, G] f32.

    Chunk q holds windows v=2q (cols 0:256) and v=2q+1 (cols 256:512);
    window strip layout: partitions 32*strip..+16 = genes 64v+16*strip..+16.
    """
    out = np.empty((B, G), np.float32)
    for c, r in enumerate(results):
        oc = np.asarray(r["out"])                          # [NC, 128, 512]
        oc = oc.reshape(NC, 4, 32, 2, 256)[:, :, :16, :, :]  # [q, strip, j, h, b]
        genes = oc.transpose(0, 3, 1, 2, 4).reshape(GP, 256)[:GC]
        out[:, c * GC:(c + 1) * GC] = genes.T
    return out


def kernel(features, tf_idx, W1, b1, Wm, bm, Wf, bf):
    from concourse.bass_utils import run_bass_kernel_spmd

    features = np.asarray(features, np.float32)
    tf_idx = np.asarray(tf_idx)
    assert not np.any(np.asarray(b1)) and not np.any(np.asarray(bm)) \
        and not np.any(np.asarray(bf)), "nonzero biases not supported"

    if "nc" not in _CACHE:
        _CACHE["nc"] = _build()
    nc = _CACHE["nc"]

    in_maps = _prepare_core_inputs(
        features, tf_idx, np.asarray(W1, np.float32),
        np.asarray(Wm, np.float32), np.asarray(Wf, np.float32))

    res = run_bass_kernel_spmd(nc, in_maps, list(range(NCORES)))
    return _assemble(res.results)
